# revision 7
# baseline (speedup 1.0000x reference)
"""MoE kernel for Trainium2 (8 NeuronCores, balanced expert-parallel).

Contract: kernel(**inputs) takes FULL unsharded inputs, returns FULL output.

Strategy: the tiny gate runs on host CPU with the exact reference ops (part of
input sharding -- token routing determines the shard). Each expert's token
set is split into 8 equal parts, one per core, so all cores carry identical
per-expert sub-batch capacities (near-perfect load balance, ~2050/16384-pair
ideal vs the 2x skew of whole-expert sharding). Each core loops over the 8
experts: stream that expert's weights from HBM (double-buffered), run
y = gelu(x @ w1[e] + b1[e]) @ w2[e] + b2[e] on its sub-batch in bf16 (fp32
accumulate), feature-major layout so no on-device transposes are needed:
  layer1: hT[h, t] += w1[d, h].T @ xT[d, t]   (lhsT = w1 tile, rhs = xT tile)
  layer2: yT[o, t] += w2[h, o].T @ hT[h, t]   (lhsT = w2 tile, rhs = hT tile)
Host scatter-adds the topw-weighted expert outputs into the full [N, O] out.
"""

import numpy as np
import ml_dtypes

N, D, O, E, TOPK = 8192, 1024, 1024, 8, 2
H = 2 * O
NCORES = 8

_BUILD_CACHE = {}
LAST_RESULTS = None  # BassKernelResults of the most recent device run


def _ensure_ntff_hook():
    """Register the axon NTFF profile hook if this image's antenv lacks it.

    concourse's run_bass_kernel_spmd(trace=True) imports
    antenv.axon_hooks.get_axon_ntff_profile_hook; on images where antenv has
    no axon_hooks submodule that import crashes. Synthesize the module from
    the boot shim's ctypes implementation when possible; degrade silently.
    """
    import sys
    import types

    try:
        import antenv.axon_hooks  # noqa: F401

        return
    except ImportError:
        pass
    try:
        import antenv
        import trn_agent_boot.trn_boot as tb

        hook = tb._ntff_profile_via_ctypes("/opt/axon/libaxon_pjrt.so")
        mod = types.ModuleType("antenv.axon_hooks")
        state = {"hook": hook}
        mod.get_axon_ntff_profile_hook = lambda: state["hook"]
        mod.set_axon_ntff_profile_hook = lambda h: state.update(hook=h)
        antenv.axon_hooks = mod
        sys.modules["antenv.axon_hooks"] = mod
    except Exception:
        pass


_ensure_ntff_hook()


def _build(caps, use_b1, use_b2):
    """caps: per-expert sub-batch capacity tuple (len E), each <= 512."""
    import concourse.bacc as bacc
    import concourse.mybir as mybir
    from concourse import tile
    from contextlib import ExitStack

    fp32 = mybir.dt.float32
    bf16 = mybir.dt.bfloat16
    AF = mybir.ActivationFunctionType

    captot = sum(caps)
    nc = bacc.Bacc(
        "TRN2", target_bir_lowering=False, debug=False, num_devices=NCORES
    )
    xt = nc.dram_tensor("xt", [D, captot], bf16, kind="ExternalInput")
    w1 = nc.dram_tensor("w1", [E * D, H], bf16, kind="ExternalInput")
    w2 = nc.dram_tensor("w2", [E * H, O], bf16, kind="ExternalInput")
    if use_b1:
        b1 = nc.dram_tensor("b1", [E * H, 1], fp32, kind="ExternalInput")
    if use_b2:
        b2 = nc.dram_tensor("b2", [E * O, 1], fp32, kind="ExternalInput")
    yt = nc.dram_tensor("yt", [O, captot], fp32, kind="ExternalOutput")

    PD, PH, PO = D // 128, H // 128, O // 128

    with ExitStack() as ctx:
        tc = ctx.enter_context(tile.TileContext(nc))
        w1pool = ctx.enter_context(tc.tile_pool(name="w1p", bufs=2))
        w2pool = ctx.enter_context(tc.tile_pool(name="w2p", bufs=1))
        bpool = ctx.enter_context(tc.tile_pool(name="bp", bufs=2))
        xpool = ctx.enter_context(tc.tile_pool(name="x", bufs=2))
        hpool = ctx.enter_context(tc.tile_pool(name="h", bufs=2))
        ypool = ctx.enter_context(tc.tile_pool(name="y", bufs=2))
        pspool = ctx.enter_context(tc.tile_pool(name="ps", bufs=4, space="PSUM"))

        off = 0
        for e in range(E):
            S = caps[e]
            # stream this expert's weights
            w1s = []
            for kd in range(PD):
                tw = w1pool.tile([128, H], bf16, tag=f"w1_{kd}")
                nc.sync.dma_start(tw[:], w1[e * D + kd * 128:e * D + (kd + 1) * 128, :])
                w1s.append(tw)
            w2s = []
            for kh in range(PH):
                tw = w2pool.tile([128, O], bf16, tag=f"w2_{kh}")
                nc.sync.dma_start(tw[:], w2[e * H + kh * 128:e * H + (kh + 1) * 128, :])
                w2s.append(tw)
            b1t = b2t = None
            if use_b1:
                b1t = bpool.tile([128, PH], fp32, tag="b1")
                for hb in range(PH):
                    nc.sync.dma_start(
                        b1t[:, hb:hb + 1],
                        b1[e * H + hb * 128:e * H + (hb + 1) * 128, :],
                    )
            if use_b2:
                b2t = bpool.tile([128, PO], fp32, tag="b2")
                for ob in range(PO):
                    nc.sync.dma_start(
                        b2t[:, ob:ob + 1],
                        b2[e * O + ob * 128:e * O + (ob + 1) * 128, :],
                    )

            xts = []
            for kd in range(PD):
                tx = xpool.tile([128, S], bf16, tag=f"x_{kd}")
                nc.gpsimd.dma_start(tx[:], xt[kd * 128:(kd + 1) * 128, off:off + S])
                xts.append(tx)

            hs = []
            for hb in range(PH):
                ps = pspool.tile([128, S], fp32, tag="ps")
                for kd in range(PD):
                    nc.tensor.matmul(
                        ps[:],
                        w1s[kd][:, hb * 128:(hb + 1) * 128],
                        xts[kd][:],
                        start=(kd == 0),
                        stop=(kd == PD - 1),
                    )
                th = hpool.tile([128, S], bf16, tag=f"h_{hb}")
                if use_b1:
                    nc.scalar.activation(th[:], ps[:], AF.Gelu, bias=b1t[:, hb:hb + 1])
                else:
                    nc.scalar.activation(th[:], ps[:], AF.Gelu)
                hs.append(th)

            for ob in range(PO):
                ps = pspool.tile([128, S], fp32, tag="ps")
                for kh in range(PH):
                    nc.tensor.matmul(
                        ps[:],
                        w2s[kh][:, ob * 128:(ob + 1) * 128],
                        hs[kh][:],
                        start=(kh == 0),
                        stop=(kh == PH - 1),
                    )
                ty = ypool.tile([128, S], fp32, tag="y")
                if use_b2:
                    nc.scalar.activation(ty[:], ps[:], AF.Copy, bias=b2t[:, ob:ob + 1])
                else:
                    nc.vector.tensor_copy(ty[:], ps[:])
                nc.sync.dma_start(yt[ob * 128:(ob + 1) * 128, off:off + S], ty[:])
            off += S

    nc.compile()
    return nc


def _gate_cpu(x, gw1, gb1, gw2, gb2):
    """Replicate the reference gate exactly (jax ops, CPU) -> topw, topi."""
    import jax
    import jax.numpy as jnp

    cpu = jax.devices("cpu")[0]
    with jax.default_device(cpu):
        xj = jnp.asarray(x)
        g = jax.nn.gelu(
            xj @ jnp.asarray(gw1) + jnp.asarray(gb1), approximate=False
        ) @ jnp.asarray(gw2) + jnp.asarray(gb2)
        gw = jax.nn.softmax(g, axis=-1)
        topw, topi = jax.lax.top_k(gw, TOPK)
        topw = topw / jnp.sum(topw, axis=-1, keepdims=True)
        return np.asarray(topw, np.float32), np.asarray(topi)


def kernel(x, gate_w1, gate_b1, gate_w2, gate_b2, w1, b1, w2, b2):
    global LAST_RESULTS
    import os
    from concourse.bass_utils import run_bass_kernel_spmd

    x = np.asarray(x, np.float32)
    w1 = np.asarray(w1, np.float32)
    b1 = np.asarray(b1, np.float32)
    w2 = np.asarray(w2, np.float32)
    b2 = np.asarray(b2, np.float32)

    topw, topi = _gate_cpu(
        x,
        np.asarray(gate_w1, np.float32),
        np.asarray(gate_b1, np.float32),
        np.asarray(gate_w2, np.float32),
        np.asarray(gate_b2, np.float32),
    )

    # token lists + gate weights per expert
    idxs, wgts = [], []
    for e in range(E):
        m0 = topi[:, 0] == e
        m1 = topi[:, 1] == e
        idx = np.nonzero(m0 | m1)[0]
        w = np.where(m0[idx], topw[idx, 0], topw[idx, 1]).astype(np.float32)
        idxs.append(idx)
        wgts.append(w)

    # split each expert across all cores equally -> per-core sub-batch sizes
    # differ by <=1; static capacity = padded max part size
    parts = [np.array_split(idxs[e], NCORES) for e in range(E)]
    wparts = [np.array_split(wgts[e], NCORES) for e in range(E)]
    caps = tuple(
        max(32, -(-max(len(p) for p in parts[e]) // 32) * 32) for e in range(E)
    )
    assert all(c <= 512 for c in caps), caps
    captot = sum(caps)
    offs = np.cumsum([0] + list(caps))[:-1]

    use_b1 = bool(np.any(b1))
    use_b2 = bool(np.any(b2))
    key = (caps, use_b1, use_b2)
    if key not in _BUILD_CACHE:
        _BUILD_CACHE[key] = _build(caps, use_b1, use_b2)
    nc = _BUILD_CACHE[key]

    bf = ml_dtypes.bfloat16
    w1b = np.ascontiguousarray(w1.reshape(E * D, H)).astype(bf)
    w2b = np.ascontiguousarray(w2.reshape(E * H, O)).astype(bf)
    if use_b1:
        b1c = np.ascontiguousarray(b1.reshape(E * H, 1), dtype=np.float32)
    if use_b2:
        b2c = np.ascontiguousarray(b2.reshape(E * O, 1), dtype=np.float32)

    in_maps = []
    for c in range(NCORES):
        xtc = np.zeros((D, captot), dtype=bf)
        for e in range(E):
            p = parts[e][c]
            if len(p):
                xtc[:, offs[e]:offs[e] + len(p)] = x[p].T.astype(bf)
        im = {"xt": xtc, "w1": w1b, "w2": w2b}
        if use_b1:
            im["b1"] = b1c
        if use_b2:
            im["b2"] = b2c
        in_maps.append(im)

    trace = bool(os.environ.get("KERNEL_TRACE"))
    LAST_RESULTS = run_bass_kernel_spmd(
        nc, in_maps, list(range(NCORES)), trace=trace
    )
    res = LAST_RESULTS.results

    out = np.zeros((N, O), np.float32)
    for c in range(NCORES):
        ytc = res[c]["yt"]
        for e in range(E):
            p = parts[e][c]
            if len(p):
                out[p] += wparts[e][c][:, None] * ytc[:, offs[e]:offs[e] + len(p)].T
    return out
